# revision 5
# baseline (speedup 1.0000x reference)
"""Trainium2 Bass kernel for nn_A2C GNN message passing (8 NeuronCores).

Strategy: GCN aggregation is linear per-channel, so both actor & critic convs
share ONE edge aggregation S0 = segment_sum(dinv[src]*x[src], dst); the convs
become S0 @ W.T applied post-aggregation.  Edges are sharded by dst range
(12544 nodes/core, no S0 all-reduce needed), sub-sharded into 8 src-chunk
bands (16 SBUF partitions each).  ap_gather fetches messages from a
transposed bf16 node table; a selector-reset prefix scan (DVE) produces
per-dst-run segment sums; boundary gathers + a [128->16] band-sum matmul
assemble S0.  deg comes from host-shipped CSR rowptr diffs; dinv is
all-gathered; epilogue (convs, MLP heads, softplus, critic reduce+MLP)
runs in channel-on-partition orientation on PE/ACT/DVE.
"""
import sys
import numpy as np

for _p in ("/opt/trn_rl_repo", "/root/.axon_site/_ro/trn_rl_repo"):
    if _p not in sys.path:
        sys.path.insert(0, _p)

import ml_dtypes

# ---------------- constants (hardcoded problem shape) ----------------
N = 100000
C = 21
H = 32
E = 6400000
NCORES = 8
NCORE = 12544            # dst nodes per core;  8*12544 = 100352 = NP
NP = NCORES * NCORE
NB = 8                   # bands (src chunks) per core
SPC = 28                 # steps per core
G = 448                  # nodes per step  (28*448 = 12544)
M = 4096                 # edge slots per (band, step) cell
TAB = NCORE + 2          # gather table elems (zero row at NCORE)
CPERM = [2 * (r % 16) + (r // 16) for r in range(32)]  # epilogue row -> channel

_CACHE = {}


# ---------------- host-side prep ----------------
def _prep_edges(edge_index):
    src = np.asarray(edge_index[0], dtype=np.int64)
    dst = np.asarray(edge_index[1], dtype=np.int64)
    nE = src.shape[0]
    core = dst // NCORE
    band = src // NCORE
    l = dst % NCORE
    step = l // G
    g = l % G
    cell = (core * NB + band) * SPC + step          # [0, 1792)
    NCELL = NCORES * NB * SPC
    key = cell * G + g
    order = np.argsort(key, kind="stable")
    cell_s = cell[order]
    g_s = g[order]
    counts = np.bincount(cell, minlength=NCELL)
    assert counts.max() <= M - 1, f"cell overflow {counts.max()}"
    starts = np.zeros(NCELL, np.int64)
    starts[1:] = np.cumsum(counts)[:-1]
    pad = (M - counts).astype(np.int64)
    jw = np.arange(nE) - starts[cell_s]
    slot = pad[cell_s] + jw

    ga = np.full((NCELL, M), NCORE, np.int16)
    src_loc = (src - band * NCORE)[order].astype(np.int16)
    ga[cell_s, slot] = src_loc

    sel = np.ones((NCELL, M), np.float32)
    isstart = np.ones(nE, bool)
    isstart[1:] = (cell_s[1:] != cell_s[:-1]) | (g_s[1:] != g_s[:-1])
    sel[cell_s[isstart], slot[isstart]] = 0.0
    sel[:, 0] = 0.0

    nodekey = cell * G + g
    ncnt = np.bincount(nodekey, minlength=NCELL * G).reshape(NCELL, G)
    cum = np.cumsum(ncnt, axis=1)
    ends = pad[:, None] + cum - 1
    ends = np.where(ncnt > 0, ends, (pad - 1)[:, None])
    bnd = ends.astype(np.int16)                      # [NCELL, G]

    # per (core, band) CSR rowptr over local nodes l (= step*G + g)
    cnt_cb = ncnt.reshape(NCORES, NB, SPC, G).reshape(NCORES, NB, NCORE)
    cumin = np.cumsum(cnt_cb, axis=2).astype(np.float32)
    cumex = cumin - cnt_cb
    return ga, sel, bnd, cumex, cumin


def _wrap16(a):
    # [NB, SPC, X*16] -> [128, SPC, X] with partition 16b+p <- col c*16+p
    nb, s, x16 = a.shape
    return a.reshape(nb, s, x16 // 16, 16).transpose(0, 3, 1, 2).reshape(
        nb * 16, s, x16 // 16).copy()


def _prep(inputs):
    x = np.asarray(inputs["x"], np.float32)
    ga, sel, bnd, cumex, cumin = _prep_edges(inputs["edge_index"])

    xpad = np.zeros((NP, 32), np.float32)
    xpad[:N, :C] = x
    # xt_tab[16b+p, e, j] = xpad[12544*b + e, 2p+j]
    xt = xpad.reshape(NB, NCORE, 16, 2).transpose(0, 2, 1, 3).reshape(128, NCORE, 2)
    xt_tab = np.zeros((128, TAB, 2), np.float32)
    xt_tab[:, :NCORE] = xt

    selmat = np.zeros((128, 16), np.float32)
    selmat[np.arange(128), np.arange(128) % 16] = 1.0
    repsel = np.zeros((8, 128), ml_dtypes.bfloat16)
    for b in range(8):
        repsel[b, 16 * b:16 * b + 16] = 1.0
    repself = repsel.astype(np.float32)
    ones1 = np.ones((1, 32), np.float32)

    def linw(w, bperm_out, perm_in):
        # lhsT[k, m] = w[rowmap(m), colmap(k)]
        K, Mo = 32, 32
        out = np.zeros((K, Mo), np.float32)
        for k in range(K):
            ck = perm_in[k] if perm_in is not None else k
            if perm_in is not None and ck >= w.shape[1]:
                continue
            if ck >= w.shape[1]:
                continue
            for m in range(Mo):
                cm = bperm_out[m] if bperm_out is not None else m
                if cm >= w.shape[0]:
                    continue
                out[k, m] = w[cm, ck]
        return out.astype(ml_dtypes.bfloat16)

    def biasv(b, perm_out):
        out = np.zeros((32, 1), np.float32)
        for m in range(32):
            cm = perm_out[m] if perm_out is not None else m
            if cm < b.shape[0]:
                out[m, 0] = b[cm]
        return out

    wmap = {}
    for pre in ("a", "c"):
        wmap[f"{pre}convw"] = linw(np.asarray(inputs[f"{pre}_convW"], np.float32), CPERM, CPERM)
        wmap[f"{pre}convb"] = biasv(np.asarray(inputs[f"{pre}_convb"], np.float32), CPERM)
        wmap[f"{pre}w1"] = linw(np.asarray(inputs[f"{pre}_w1"], np.float32), None, CPERM)
        wmap[f"{pre}b1"] = biasv(np.asarray(inputs[f"{pre}_b1"], np.float32), None)
        wmap[f"{pre}w2"] = linw(np.asarray(inputs[f"{pre}_w2"], np.float32), None, None)
        wmap[f"{pre}b2"] = biasv(np.asarray(inputs[f"{pre}_b2"], np.float32), None)
        w3 = np.zeros((32, 1), np.float32)
        w3[:H, 0] = np.asarray(inputs[f"{pre}_w3"], np.float32)[0]
        wmap[f"{pre}w3"] = w3.astype(ml_dtypes.bfloat16)
        wmap[f"{pre}b3"] = np.asarray(inputs[f"{pre}_b3"], np.float32).reshape(1, 1)

    ga4 = ga.reshape(NCORES, NB, SPC, M)
    sel4 = sel.reshape(NCORES, NB, SPC, M)
    bnd4 = bnd.reshape(NCORES, NB, SPC, G)

    in_maps = []
    for k in range(NCORES):
        lo = k * NCORE
        xt_loc = np.zeros((32, NCORE), np.float32)
        for r in range(32):
            if CPERM[r] < 32:
                xt_loc[r] = xpad[lo:lo + NCORE, CPERM[r]]
        mask = np.zeros((32, NCORE), np.float32)
        nreal = min(max(N - lo, 0), NCORE)
        mask[:, :nreal] = 1.0
        se = np.stack([cumex[k], cumin[k]], axis=1)      # [NB, 2, NCORE]
        se_w = se.reshape(NB, 2, 16, 784).transpose(0, 2, 1, 3).reshape(128, 2, 784)
        m = dict(
            xt_tab=xt_tab,
            xt_loc=xt_loc,
            mask=mask,
            sel8=sel4[k].astype(ml_dtypes.bfloat16),     # [NB, SPC, M]
            ga_idx=_wrap16(ga4[k]),                      # [128, SPC, 256]
            bnd_idx=_wrap16(bnd4[k]),                    # [128, SPC, 28]
            se=se_w.astype(np.float32),
            selmat=selmat, repsel=repsel, repself=repself, ones1=ones1,
        )
        m.update(wmap)
        in_maps.append(m)
    return in_maps


# ---------------- device kernel ----------------
def _build():
    import concourse.bass as bass
    import concourse.bacc as bacc
    import concourse.tile as tile
    import concourse.mybir as mybir

    f32, bf16, i16 = mybir.dt.float32, mybir.dt.bfloat16, mybir.dt.int16
    AF = mybir.ActivationFunctionType
    OP = mybir.AluOpType

    nc = bacc.Bacc(None, target_bir_lowering=False)
    dp = nc.declare_dram_parameter
    xt_tab_d = dp("xt_tab", [128, TAB, 2], f32, isOutput=False)
    xt_loc_d = dp("xt_loc", [32, NCORE], f32, isOutput=False)
    mask_d = dp("mask", [32, NCORE], f32, isOutput=False)
    sel8_d = dp("sel8", [NB, SPC, M], bf16, isOutput=False)
    ga_d = dp("ga_idx", [128, SPC, M // 16], i16, isOutput=False)
    bnd_d = dp("bnd_idx", [128, SPC, G // 16], i16, isOutput=False)
    se_d = dp("se", [128, 2, 784], f32, isOutput=False)
    selmat_d = dp("selmat", [128, 16], f32, isOutput=False)
    repsel_d = dp("repsel", [8, 128], bf16, isOutput=False)
    repself_d = dp("repself", [8, 128], f32, isOutput=False)
    ones1_d = dp("ones1", [1, 32], f32, isOutput=False)
    wd = {}
    for pre in ("a", "c"):
        wd[f"{pre}convw"] = dp(f"{pre}convw", [32, 32], bf16, isOutput=False)
        wd[f"{pre}convb"] = dp(f"{pre}convb", [32, 1], f32, isOutput=False)
        wd[f"{pre}w1"] = dp(f"{pre}w1", [32, 32], bf16, isOutput=False)
        wd[f"{pre}b1"] = dp(f"{pre}b1", [32, 1], f32, isOutput=False)
        wd[f"{pre}w2"] = dp(f"{pre}w2", [32, 32], bf16, isOutput=False)
        wd[f"{pre}b2"] = dp(f"{pre}b2", [32, 1], f32, isOutput=False)
        wd[f"{pre}w3"] = dp(f"{pre}w3", [32, 1], bf16, isOutput=False)
        wd[f"{pre}b3"] = dp(f"{pre}b3", [1, 1], f32, isOutput=False)
    conc_d = dp("conc", [1, NCORE], f32, isOutput=True)
    val_d = dp("val", [1, 1], f32, isOutput=True)

    s0_dram = nc.dram_tensor("s0_dram", [32, NCORE], f32)
    dinv_dram = nc.dram_tensor("dinv_dram", [16, 784], f32)

    with tile.TileContext(nc) as tc:
        with (tc.tile_pool(name="persist", bufs=1) as pp,
              tc.tile_pool(name="psum", bufs=2, space="PSUM") as psp,
              tc.tile_pool(name="dram", bufs=1, space="DRAM") as dpool):
            t_tab = pp.tile([128, TAB, 2], bf16)
            ga_sb = pp.tile([128, SPC, M // 16], i16)
            bnd_sb = pp.tile([128, SPC, G // 16], i16)
            selmat_sb = pp.tile([128, 16], f32)
            repsel_sb = pp.tile([8, 128], bf16)
            nc.sync.dma_start(ga_sb[:], ga_d[:])
            nc.sync.dma_start(bnd_sb[:], bnd_d[:])
            nc.sync.dma_start(selmat_sb[:], selmat_d[:])
            nc.sync.dma_start(repsel_sb[:], repsel_d[:])

            # ---- deg -> dinv (local dst range) ----
            with tc.tile_pool(name="degp", bufs=1) as dgp:
                se_sb = dgp.tile([128, 2, 784], f32)
                nc.sync.dma_start(se_sb[:], se_d[:])
                dif = dgp.tile([128, 784], f32)
                nc.vector.tensor_tensor(out=dif[:], in0=se_sb[:, 1, :],
                                        in1=se_sb[:, 0, :], op=OP.subtract)
                deg16 = dgp.tile([16, 784], f32)
                for h in range(2):
                    cs = slice(h * 392, h * 392 + 392)
                    psd = psp.tile([16, 392], f32, space="PSUM", tag="psd")
                    nc.tensor.matmul(psd[:], selmat_sb[:], dif[:, cs],
                                     start=True, stop=True)
                    nc.scalar.activation(deg16[:, cs], psd[:], AF.Copy)
                degp1 = dgp.tile([16, 784], f32)
                nc.vector.tensor_scalar(out=degp1[:], in0=deg16[:], scalar1=1.0,
                                        scalar2=None, op0=OP.add)
                rec = dgp.tile([16, 784], f32)
                nc.vector.reciprocal(rec[:], degp1[:])
                dinv16 = dgp.tile([16, 784], f32)
                nc.scalar.activation(dinv16[:], rec[:], AF.Sqrt)
                nc.sync.dma_start(dinv_dram[:], dinv16[:])

            # ---- AllGather dinv across cores ----
            ag_in = dpool.tile([16, 784], f32)
            ag_out = dpool.tile([128, 784], f32)
            nc.sync.dma_start(ag_in[:], dinv_dram[:])
            nc.gpsimd.collective_compute(
                "AllGather", mybir.AluOpType.bypass,
                replica_groups=[list(range(NCORES))],
                ins=[ag_in[:].opt()], outs=[ag_out[:].opt()])

            # ---- build gather table: t_tab = bf16(xt_tab * dinv_rep) ----
            with tc.tile_pool(name="tbp", bufs=1) as tbp:
                repself_sb = tbp.tile([8, 128], f32)
                nc.sync.dma_start(repself_sb[:], repself_d[:])
                dinv8 = tbp.tile([8, NCORE], f32)
                nc.sync.dma_start(dinv8[:], ag_out[:].rearrange("(a b) c -> a (b c)", a=8))
                dinv_rep = tbp.tile([128, NCORE], f32)
                for h in range(NCORE // 448):
                    cs = slice(h * 448, h * 448 + 448)
                    psr = psp.tile([128, 448], f32, space="PSUM", tag="psr")
                    nc.tensor.matmul(psr[:], repself_sb[:], dinv8[:, cs],
                                     start=True, stop=True)
                    nc.scalar.activation(dinv_rep[:, cs], psr[:], AF.Copy)
                for h in range(4):
                    cs = slice(h * 3136, h * 3136 + 3136)
                    xch = tbp.tile([128, 3136, 2], f32, tag="xch")
                    nc.sync.dma_start(xch[:], xt_tab_d[:, cs, :])
                    nc.vector.tensor_tensor(
                        out=t_tab[:, cs, :], in0=xch[:],
                        in1=dinv_rep[:, cs].rearrange("a b -> a b ()").to_broadcast([128, 3136, 2]),
                        op=OP.mult)
                nc.vector.memset(t_tab[:, NCORE:TAB, :], 0)

            # ---- main loop over steps ----
            with tc.tile_pool(name="mainp", bufs=2) as mp:
                for c in range(SPC):
                    sel8_sb = mp.tile([8, M], bf16, tag="sel8")
                    nc.sync.dma_start(sel8_sb[:], sel8_d[:, c, :])
                    sel_sb = mp.tile([128, M], bf16, tag="sel")
                    for h in range(M // 512):
                        cs = slice(h * 512, h * 512 + 512)
                        pss = psp.tile([128, 512], f32, space="PSUM", tag="pss")
                        nc.tensor.matmul(pss[:], repsel_sb[:], sel8_sb[:, cs],
                                         start=True, stop=True)
                        nc.scalar.activation(sel_sb[:, cs], pss[:], AF.Copy)
                    msgs = mp.tile([128, M, 2], bf16, tag="msgs")
                    nc.gpsimd.ap_gather(msgs[:], t_tab[:], ga_sb[:, c, :],
                                        channels=128, num_elems=TAB, d=2, num_idxs=M)
                    P = mp.tile([128, M, 2], f32, tag="P")
                    for j in range(2):
                        nc.vector.tensor_tensor_scan(
                            P[:, :, j], sel_sb[:], msgs[:, :, j], 0.0,
                            OP.mult, OP.add)
                    Bnd = mp.tile([128, G, 2], f32, tag="Bnd")
                    nc.gpsimd.ap_gather(Bnd[:], P[:], bnd_sb[:, c, :],
                                        channels=128, num_elems=M, d=2, num_idxs=G)
                    for j in range(2):
                        psb = psp.tile([16, G], f32, space="PSUM", tag="psb")
                        nc.tensor.matmul(psb[:], selmat_sb[:], Bnd[:, :, j],
                                         start=True, stop=True)
                        s0j = mp.tile([16, G], f32, tag="s0j")
                        nc.scalar.activation(s0j[:], psb[:], AF.Copy)
                        nc.sync.dma_start(
                            s0_dram[16 * j:16 * j + 16, c * G:(c + 1) * G], s0j[:])

        # ---- epilogue ----
        with (tc.tile_pool(name="epi", bufs=1) as ep,
              tc.tile_pool(name="psume", bufs=1, space="PSUM") as pse,
              tc.tile_pool(name="drame", bufs=1, space="DRAM") as dpe):
            w_sb = {}
            for key, dram in wd.items():
                wt = ep.tile(list(dram.shape), dram.dtype, name=f"w_{key}", tag=f"w_{key}")
                nc.sync.dma_start(wt[:], dram[:])
                w_sb[key] = wt
            ones1_sb = ep.tile([1, 32], f32)
            nc.sync.dma_start(ones1_sb[:], ones1_d[:])
            csum_acc = ep.tile([32, 1], f32)
            nc.vector.memset(csum_acc[:], 0)

            NCH = 8
            CHW = NCORE // NCH          # 1568
            PW = CHW // 4               # 392
            for ci in range(NCH):
                cs = slice(ci * CHW, (ci + 1) * CHW)
                s0t = ep.tile([32, CHW], f32, tag="s0t")
                nc.sync.dma_start(s0t[:], s0_dram[:, cs])
                xt = ep.tile([32, CHW], f32, tag="xt")
                nc.sync.dma_start(xt[:], xt_loc_d[:, cs])
                mk = ep.tile([32, CHW], f32, tag="mk")
                nc.sync.dma_start(mk[:], mask_d[:, cs])
                dv1 = ep.tile([1, CHW], f32, tag="dv1")
                nc.sync.dma_start(
                    dv1[:], dinv_dram[:].rearrange("a b -> () (a b)")[:, cs])
                dve = ep.tile([32, CHW], f32, tag="dve")
                for h in range(CHW // PW):
                    hs = slice(h * PW, h * PW + PW)
                    psv = pse.tile([32, PW], f32, space="PSUM", tag="pe32")
                    nc.tensor.matmul(psv[:], ones1_sb[:], dv1[:, hs],
                                     start=True, stop=True)
                    nc.scalar.activation(dve[:, hs], psv[:], AF.Copy)
                # S = dinv*S0 + dinv^2*x
                t1 = ep.tile([32, CHW], f32, tag="t1")
                nc.vector.tensor_tensor(out=t1[:], in0=s0t[:], in1=dve[:], op=OP.mult)
                t2 = ep.tile([32, CHW], f32, tag="t2")
                nc.vector.tensor_tensor(out=t2[:], in0=xt[:], in1=dve[:], op=OP.mult)
                nc.vector.tensor_tensor(out=t2[:], in0=t2[:], in1=dve[:], op=OP.mult)
                Sb = ep.tile([32, CHW], bf16, tag="Sb")
                nc.vector.tensor_tensor(out=Sb[:], in0=t1[:], in1=t2[:], op=OP.add)

                for pre in ("a", "c"):
                    conv = ep.tile([32, CHW], f32, tag=f"conv{pre}")
                    for h in range(CHW // PW):
                        hs = slice(h * PW, h * PW + PW)
                        psc = pse.tile([32, PW], f32, space="PSUM", tag="pe32b")
                        nc.tensor.matmul(psc[:], w_sb[f"{pre}convw"][:], Sb[:, hs],
                                         start=True, stop=True)
                        nc.scalar.activation(conv[:, hs], psc[:], AF.Relu,
                                             bias=w_sb[f"{pre}convb"][:])
                    act = ep.tile([32, CHW], bf16 if pre == "a" else f32,
                                  name=f"act{pre}", tag=f"act{pre}")
                    nc.vector.tensor_tensor(out=act[:], in0=conv[:], in1=xt[:], op=OP.add)
                    if pre == "a":
                        h1 = ep.tile([32, CHW], bf16, tag="h1")
                        h2 = ep.tile([32, CHW], bf16, tag="h2")
                        for h in range(CHW // PW):
                            hs = slice(h * PW, h * PW + PW)
                            ps1 = pse.tile([32, PW], f32, space="PSUM", tag="pe32c")
                            nc.tensor.matmul(ps1[:], w_sb["aw1"][:], act[:, hs],
                                             start=True, stop=True)
                            nc.scalar.activation(h1[:, hs], ps1[:], AF.Relu,
                                                 bias=w_sb["ab1"][:])
                        for h in range(CHW // PW):
                            hs = slice(h * PW, h * PW + PW)
                            ps2 = pse.tile([32, PW], f32, space="PSUM", tag="pe32d")
                            nc.tensor.matmul(ps2[:], w_sb["aw2"][:], h1[:, hs],
                                             start=True, stop=True)
                            nc.scalar.activation(h2[:, hs], ps2[:], AF.Relu,
                                                 bias=w_sb["ab2"][:])
                        zt = ep.tile([1, CHW], f32, tag="zt")
                        for h in range(CHW // PW):
                            hs = slice(h * PW, h * PW + PW)
                            ps3 = pse.tile([1, PW], f32, space="PSUM", tag="pe1")
                            nc.tensor.matmul(ps3[:], w_sb["aw3"][:], h2[:, hs],
                                             start=True, stop=True)
                            nc.vector.tensor_scalar(out=zt[:, hs], in0=ps3[:],
                                                    scalar1=w_sb["ab3"][:],
                                                    scalar2=None, op0=OP.add)
                        # stable softplus: relu(z) + ln(1 + exp(-|z|))
                        azt = ep.tile([1, CHW], f32, tag="azt")
                        nc.scalar.activation(azt[:], zt[:], AF.Abs)
                        ezt = ep.tile([1, CHW], f32, tag="ezt")
                        nc.scalar.activation(ezt[:], azt[:], AF.Exp, scale=-1.0)
                        lzt = ep.tile([1, CHW], f32, tag="lzt")
                        nc.scalar.activation(lzt[:], ezt[:], AF.Ln, bias=1.0)
                        rzt = ep.tile([1, CHW], f32, tag="rzt")
                        nc.scalar.activation(rzt[:], zt[:], AF.Relu)
                        conc_sb = ep.tile([1, CHW], f32, tag="conc_sb")
                        nc.vector.tensor_tensor(out=conc_sb[:], in0=rzt[:],
                                                in1=lzt[:], op=OP.add)
                        nc.sync.dma_start(conc_d[:, cs], conc_sb[:])
                    else:
                        cm = ep.tile([32, CHW], f32, tag="cm")
                        nc.vector.tensor_tensor(out=cm[:], in0=act[:], in1=mk[:],
                                                op=OP.mult)
                        cred = ep.tile([32, 1], f32, tag="cred")
                        nc.vector.tensor_reduce(cred[:], cm[:],
                                                axis=mybir.AxisListType.X, op=OP.add)
                        nc.vector.tensor_tensor(out=csum_acc[:], in0=csum_acc[:],
                                                in1=cred[:], op=OP.add)

            # critic value via AllReduce of csum
            cs_in = dpe.tile([32, 1], f32)
            cs_out = dpe.tile([32, 1], f32)
            nc.sync.dma_start(cs_in[:], csum_acc[:])
            nc.gpsimd.collective_compute(
                "AllReduce", mybir.AluOpType.add,
                replica_groups=[list(range(NCORES))],
                ins=[cs_in[:].opt()], outs=[cs_out[:].opt()])
            csf = ep.tile([32, 1], f32)
            nc.sync.dma_start(csf[:], cs_out[:])
            csg = ep.tile([32, 1], bf16)
            nc.vector.tensor_copy(csg[:], csf[:])
            ch1 = ep.tile([32, 1], bf16)
            ch2 = ep.tile([32, 1], bf16)
            psc1 = pse.tile([32, 1], f32, space="PSUM", tag="pec")
            nc.tensor.matmul(psc1[:], w_sb["cw1"][:], csg[:], start=True, stop=True)
            nc.scalar.activation(ch1[:], psc1[:], AF.Relu, bias=w_sb["cb1"][:])
            psc2 = pse.tile([32, 1], f32, space="PSUM", tag="pecb")
            nc.tensor.matmul(psc2[:], w_sb["cw2"][:], ch1[:], start=True, stop=True)
            nc.scalar.activation(ch2[:], psc2[:], AF.Relu, bias=w_sb["cb2"][:])
            psc3 = pse.tile([1, 1], f32, space="PSUM", tag="pe1")
            nc.tensor.matmul(psc3[:], w_sb["cw3"][:], ch2[:], start=True, stop=True)
            val_sb = ep.tile([1, 1], f32)
            nc.vector.tensor_tensor(out=val_sb[:], in0=psc3[:], in1=w_sb["cb3"][:],
                                    op=OP.add)
            nc.sync.dma_start(val_d[:], val_sb[:])

    nc.finalize()
    return nc


def _make_runner(nc, n_cores=8):
    import jax
    import numpy as _np
    from jax.sharding import Mesh, PartitionSpec, NamedSharding
    from jax.experimental.shard_map import shard_map
    import concourse.mybir as mybir
    from concourse import bass2jax

    bass2jax.install_neuronx_cc_hook()
    partition_name = nc.partition_id_tensor.name if nc.partition_id_tensor else None
    in_names, out_names, out_avals = [], [], []
    for alloc in nc.m.functions[0].allocations:
        if not isinstance(alloc, mybir.MemoryLocationSet):
            continue
        name = alloc.memorylocations[0].name
        if alloc.kind == "ExternalInput":
            if name != partition_name:
                in_names.append(name)
        elif alloc.kind == "ExternalOutput":
            out_avals.append(jax.core.ShapedArray(tuple(alloc.tensor_shape),
                                                  mybir.dt.np(alloc.dtype)))
            out_names.append(name)
    n_params = len(in_names)
    all_in = list(in_names) + list(out_names)
    if partition_name is not None:
        all_in.append(partition_name)

    def _body(*args):
        operands = list(args)
        if partition_name is not None:
            operands.append(bass2jax.partition_id_tensor())
        outs = bass2jax._bass_exec_p.bind(
            *operands, out_avals=tuple(out_avals), in_names=tuple(all_in),
            out_names=tuple(out_names), lowering_input_output_aliases=(),
            sim_require_finite=True, sim_require_nnan=True, nc=nc)
        return tuple(outs)

    devices = jax.devices()[:n_cores]
    mesh = Mesh(_np.asarray(devices), ("core",))
    n_outs = len(out_avals)
    sharded = jax.jit(
        shard_map(_body, mesh=mesh,
                  in_specs=(PartitionSpec("core"),) * (n_params + n_outs),
                  out_specs=(PartitionSpec("core"),) * n_outs,
                  check_rep=False),
        donate_argnums=tuple(range(n_params, n_params + n_outs)),
        keep_unused=True)
    sharding = NamedSharding(mesh, PartitionSpec("core"))
    return sharded, sharding, in_names, out_names, out_avals


def _get_exec():
    if "exec" not in _CACHE:
        nc = _build()
        _CACHE["exec"] = _make_runner(nc)
    return _CACHE["exec"]


def run_device(in_maps, n_timing=1):
    """Returns (outs_per_core, wall_times). outs: list of dicts per core."""
    import jax
    import time
    sharded, sharding, in_names, out_names, out_avals = _get_exec()
    n_cores = NCORES
    concat_in = [np.concatenate([np.ascontiguousarray(in_maps[c][k])
                                 for c in range(n_cores)], axis=0)
                 for k in in_names]
    dev_in = [jax.device_put(a, sharding) for a in concat_in]
    for a in dev_in:
        a.block_until_ready()
    walls, outs = [], None
    for _ in range(n_timing):
        zeros = [jax.device_put(
            np.zeros((n_cores * av.shape[0], *av.shape[1:]), av.dtype), sharding)
            for av in out_avals]
        for z in zeros:
            z.block_until_ready()
        t0 = time.perf_counter()
        outs = sharded(*dev_in, *zeros)
        for o in outs:
            o.block_until_ready()
        walls.append(time.perf_counter() - t0)
    out_np = [
        {name: np.asarray(o).reshape(n_cores, *out_avals[i].shape)[c]
         for i, (name, o) in enumerate(zip(out_names, outs))}
        for c in range(n_cores)]
    return out_np, walls


def kernel(**inputs):
    in_maps = _prep(inputs)
    outs, _ = run_device(in_maps, n_timing=1)
    conc = np.concatenate([outs[c]["conc"][0] for c in range(NCORES)])[:N]
    value = outs[0]["val"].reshape(1)
    return conc.astype(np.float32), value.astype(np.float32)
